# revision 34
# baseline (speedup 1.0000x reference)
"""Trainium2 Bass kernel for the AggregateLayer pooling problem.

reference semantics (per batch b):
    dot_w[j] = <pref[b,j,:], c[b,0,:]>                      (j = 0..63)
    t_w[j]   = 1 / |t_pref[b,0,j] - t_c[b,0]|
    w        = softmax(dot_w + t_w)                          (over j)
    u[b,0,:] = sum_j w[j] * pref[b,j,:]

Strategy: pure data parallel over 8 NeuronCores (1024 batches each).

Per core, batches stream in chunks of 64 (2 MB of fp32 pref). The chunk is
DMA'd CONTIGUOUSLY (16 KB per partition, cast fp32->fp16 in flight by
SWDGE), which puts partition p = 32 consecutive flat rows; with N=64 this
means partition p = (batch pair q=p//2, j-half h=p%2) and free t = j%32.

Engine plan per chunk:
  - GpSimd: pref cast-DMA issue; Y = P16 * c_pair (elementwise); tw chain bits
  - Vector: dots = reduce_sum_d(Y); softmax small ops (pair-merge via
    stream_shuffle); fused weight build W = (e * 1/Z) * SPAIR
  - Scalar: exp (+ per-partition sum accumulator); abs; PSUM->SBUF copies
  - Tensor: c pair-broadcast matmul; 32 accumulating weighted-sum matmuls
    contracting the partition dim with the block-pair selector inside W
No PE transposes and no full-size PSUM->SBUF copies are needed.
"""

import numpy as np
from contextlib import ExitStack

import concourse.bass as bass
import concourse.tile as tile
from concourse import mybir
from concourse.bass_utils import run_bass_kernel_spmd
import concourse.bass2jax as _b2j


def _split_multiwait(bir: dict) -> int:
    """Walrus in this container rejects >1 sync-wait per instruction.

    Hoist excess waits onto NoOps inserted just before the instruction on
    the same engine (program order within the engine stream preserves the
    wait semantics exactly).
    """
    n = 0
    for fn in bir["functions"]:
        for blk in fn["blocks"]:
            out = []
            for inst in blk["instructions"]:
                si = inst.get("sync_info")
                waits = si.get("on_wait") if si else None
                if waits and len(waits) > 1:
                    for w in waits[:-1]:
                        out.append(
                            {
                                "opcode": "NoOp",
                                "engine": inst["engine"],
                                "name": f"{inst['name']}-xw{n}",
                                "ins": [],
                                "outs": [],
                                "sync_info": {"on_update": [], "on_wait": [w]},
                            }
                        )
                        n += 1
                    si["on_wait"] = [waits[-1]]
                out.append(inst)
            blk["instructions"] = out
    return n


_orig_compile_bir_kernel = _b2j.compile_bir_kernel


def _legalizing_compile_bir_kernel(ant_bir_str, *args, **kwargs):
    import orjson

    bir = orjson.loads(ant_bir_str)
    _split_multiwait(bir)
    return _orig_compile_bir_kernel(orjson.dumps(bir), *args, **kwargs)


_b2j.compile_bir_kernel = _legalizing_compile_bir_kernel

F32 = mybir.dt.float32
F16 = mybir.dt.float16
Alu = mybir.AluOpType
Act = mybir.ActivationFunctionType
Axis = mybir.AxisListType

B, N, D = 8192, 64, 128
NCORES = 8
BPC = B // NCORES          # 1024 batches per core
CHUNK = 64                 # batches per chunk
NCHUNK = BPC // CHUNK      # 16
NT = 32                    # free positions per partition row-block (j % 32)
ROWS = CHUNK * N           # 4096 flat rows per chunk

# stream_shuffle mask: swap adjacent partitions within each 32-block
SWAPMASK = [i ^ 1 for i in range(32)]


def _stage_load(nc, pools, aps, tiles, k):
    """Issue chunk k's DMAs + fp32->fp16 cast (runs ahead of compute)."""
    (p_pref32, p_pref, p_y, p_w, p_cexp, p_sm, p_u, ps_cexp, ps_u, ps_z) = pools
    (pref_rows, tp_rows, u_rows) = aps
    r0 = k * ROWS

    p32 = p_pref32.tile([128, NT, D], F32, tag="p32")
    nc.sync.dma_start(
        out=p32[:],
        in_=pref_rows[r0 : r0 + ROWS, :].rearrange("(p t) d -> p t d", p=128),
    )
    # cast writes the (d-half, t, d-low) permuted layout so the dots
    # d-halves fold is a big-descriptor SBUF->SBUF accumulate-DMA
    p16 = p_pref.tile([128, 2, NT, D // 2], F16, tag="p16")
    nc.scalar.copy(
        out=p16[:].transpose([0, 2, 1, 3]),
        in_=p32[:].rearrange("p t (d2 d1) -> p t d2 d1", d2=2),
    )

    tp_k = p_sm.tile([128, NT], F32, tag="tpk")
    nc.sync.dma_start(
        out=tp_k[:],
        in_=tp_rows[k * CHUNK : (k + 1) * CHUNK, :].rearrange(
            "q (h t) -> (q h) t", h=2
        ),
    )
    tiles[k] = (p16, tp_k)


def _build_chunk(nc, pools, consts, aps, tiles, k):
    (p_pref32, p_pref, p_y, p_w, p_cexp, p_sm, p_u, ps_cexp, ps_u, ps_z) = pools
    (sp16, spT16, cg16, tca) = consts
    (pref_rows, tp_rows, u_rows) = aps
    (p16, tp_k) = tiles.pop(k)

    # ---- c broadcast to partition pairs via selector matmul; x128 so the
    # ---- avg-pool (sum/128) below yields the plain dot product ----------
    cexp_ps = ps_cexp.tile([128, D], F32, tag="cexp_ps")
    nc.tensor.matmul(
        out=cexp_ps[:], lhsT=spT16[:], rhs=cg16[:, k, :], start=True, stop=True
    )
    cexp16 = p_cexp.tile([128, D], F16, tag="cexp16")
    nc.scalar.copy(out=cexp16[:], in_=cexp_ps[:])

    # ---- dots: Y = P16 * c (vector); fold d-halves via accumulate-DMA;
    # ---- reduce the remaining 64 on vector -------------------------------
    y16 = p_y.tile([128, 2, NT, D // 2], F16, tag="y16")
    nc.vector.tensor_tensor(
        out=y16[:],
        in0=p16[:],
        in1=cexp16[:]
        .rearrange("p (d2 d1) -> p d2 d1", d2=2)
        .unsqueeze(2)
        .broadcast_to((128, 2, NT, D // 2)),
        op=Alu.mult,
    )
    nc.gpsimd.dma_start(
        out=y16[:, 0, :, :], in_=y16[:, 1, :, :], accum_op=Alu.add
    )
    dots = p_sm.tile([128, NT], F32, tag="dots")
    nc.vector.reduce_sum(out=dots[:], in_=y16[:, 0, :, :], axis=Axis.X)

    # ---- t_w = 1/|t_pref - t_c|  (tca holds -t_c: fused sub+abs) ---------
    adtw = p_sm.tile([128, NT], F32, tag="adtw")
    nc.scalar.activation(
        out=adtw[:], in_=tp_k[:], func=Act.Abs, bias=tca[:, k : k + 1]
    )
    tw = p_sm.tile([128, NT], F32, tag="tw")
    nc.vector.reciprocal(out=tw[:], in_=adtw[:])

    # ---- wpre = dots + t_w; nmh = -max(wpre) ----------------------------
    wpre = p_sm.tile([128, NT], F32, tag="wpre")
    nc.vector.tensor_add(out=wpre[:], in0=dots[:], in1=tw[:])
    nmh = p_sm.tile([128, 1], F32, tag="nmh")
    nc.vector.tensor_reduce(
        out=nmh[:], in_=wpre[:], axis=Axis.X, op=Alu.max, negate=True
    )

    # ---- pair-merge of -max via tiny SBUF->SBUF DMAs ---------------------
    nms = p_sm.tile([128, 1], F32, tag="nms")
    nmh_v = nmh[:].rearrange("(q h) one -> q h one", h=2)
    nms_v = nms[:].rearrange("(q h) one -> q h one", h=2)
    nc.sync.dma_start(out=nms_v[:, 0, :], in_=nmh_v[:, 1, :])
    nc.sync.dma_start(out=nms_v[:, 1, :], in_=nmh_v[:, 0, :])
    nm = p_sm.tile([128, 1], F32, tag="nm")
    nc.vector.tensor_tensor(out=nm[:], in0=nmh[:], in1=nms[:], op=Alu.min)

    # ---- e = exp(wpre - max) ---------------------------------------------
    e16 = p_sm.tile([128, NT], F16, tag="e16")
    nc.scalar.activation(
        out=e16[:], in_=wpre[:], func=Act.Exp, bias=nm[:], scale=1.0
    )

    # ---- Z[q] = sum_{p in pair q, t} e16  (PE, lands on 64 partitions) ---
    zps = ps_z.tile([CHUNK, NT], F32, tag="zps")
    nc.tensor.matmul(
        out=zps[:], lhsT=sp16[:], rhs=e16[:], start=True, stop=True
    )
    zq = p_sm.tile([CHUNK, 1], F32, tag="zq")
    nc.vector.reduce_sum(out=zq[:], in_=zps[:], axis=Axis.X)
    rzq = p_sm.tile([CHUNK, 1], F32, tag="rzq")
    nc.vector.reciprocal(out=rzq[:], in_=zq[:])

    # ---- W[p, t, q] = e16[p,t] * SPAIR[p, q]  (gpsimd) -------------------
    w16 = p_w.tile([128, NT, CHUNK], F16, tag="w16")
    nc.gpsimd.tensor_tensor(
        out=w16[:],
        in0=e16[:].unsqueeze(2).broadcast_to((128, NT, CHUNK)),
        in1=sp16[:].unsqueeze(1).broadcast_to((128, NT, CHUNK)),
        op=Alu.mult,
    )

    # ---- weighted sum: u[q, d] += sum_p W[p,t,q] * P16[p,t,d] ------------
    ups = ps_u.tile([CHUNK, D], F32, tag="ups")
    for t in range(NT):
        nc.tensor.matmul(
            out=ups[:],
            lhsT=w16[:, t, :],
            rhs=p16[:, :, t, :],
            start=(t == 0),
            stop=(t == NT - 1),
        )

    # ---- u = ups / Z, store ----------------------------------------------
    usb = p_u.tile([CHUNK, D], F32, tag="usb")
    nc.scalar.mul(out=usb[:], in_=ups[:], mul=rzq[:])
    nc.sync.dma_start(
        out=u_rows[k * CHUNK : (k + 1) * CHUNK, :], in_=usb[:]
    )


def _build_nc():
    nc = bass.Bass()
    pref = nc.declare_dram_parameter("pref", [BPC, N, D], F32, isOutput=False)
    c = nc.declare_dram_parameter("c", [BPC, 1, D], F32, isOutput=False)
    t_pref = nc.declare_dram_parameter("t_pref", [BPC, 1, N], F32, isOutput=False)
    t_c = nc.declare_dram_parameter("t_c", [BPC, 1], F32, isOutput=False)
    spair = nc.declare_dram_parameter("spair", [128, CHUNK], F16, isOutput=False)
    spairT = nc.declare_dram_parameter("spairT", [CHUNK, 128], F16, isOutput=False)
    spairT32 = nc.declare_dram_parameter(
        "spairT32", [CHUNK, 128], F32, isOutput=False
    )
    u = nc.declare_dram_parameter("u", [BPC, 1, D], F32, isOutput=True)

    pref_rows = pref[:].rearrange("b n d -> (b n) d")
    c_rows = c[:].rearrange("b one d -> (b one) d")
    tp_rows = t_pref[:].rearrange("b one n -> (b one) n")
    tc_rows = t_c[:]
    u_rows = u[:].rearrange("b one d -> (b one) d")

    with ExitStack() as ctx:
        tc = ctx.enter_context(tile.TileContext(nc))
        p_const = ctx.enter_context(tc.tile_pool(name="const", bufs=1))

        # constants / preloads (issued up front; small side-queue traffic)
        sp16 = p_const.tile([128, CHUNK], F16)
        nc.sync.dma_start(out=sp16[:], in_=spair[:])
        spT16 = p_const.tile([CHUNK, 128], F16)
        nc.sync.dma_start(out=spT16[:], in_=spairT[:])
        spT32 = p_const.tile([CHUNK, 128], F32)
        nc.sync.dma_start(out=spT32[:], in_=spairT32[:])

        cg16 = p_const.tile([CHUNK, NCHUNK, D], F16)
        nc.gpsimd.dma_start(
            out=cg16[:],
            in_=c_rows[:].rearrange("(k q) d -> q k d", q=CHUNK),
        )
        # t_c on 64 partitions, then expand to partition pairs via PE
        tc64 = p_const.tile([CHUNK, NCHUNK], F32)
        nc.sync.dma_start(
            out=tc64[:],
            in_=tc_rows[:].rearrange("(k q) one -> q (k one)", q=CHUNK),
        )
        ps_t = ctx.enter_context(tc.tile_pool(name="ps_t", bufs=1, space="PSUM"))
        tca_ps = ps_t.tile([128, NCHUNK], F32)
        nc.tensor.matmul(
            out=tca_ps[:], lhsT=spT32[:], rhs=tc64[:], start=True, stop=True
        )
        tca = p_const.tile([128, NCHUNK], F32)
        nc.scalar.copy(out=tca[:], in_=tca_ps[:])

        consts = (sp16, spT16, cg16, tca)
        aps = (pref_rows, tp_rows, u_rows)

        p_pref32 = ctx.enter_context(tc.tile_pool(name="pref32", bufs=3))
        p_pref = ctx.enter_context(tc.tile_pool(name="pref", bufs=4))
        p_y = ctx.enter_context(tc.tile_pool(name="y", bufs=3))
        p_w = ctx.enter_context(tc.tile_pool(name="w", bufs=3))
        p_cexp = ctx.enter_context(tc.tile_pool(name="cexp", bufs=3))
        p_sm = ctx.enter_context(tc.tile_pool(name="sm", bufs=6))
        p_u = ctx.enter_context(tc.tile_pool(name="u", bufs=3))
        ps_cexp = ctx.enter_context(
            tc.tile_pool(name="ps_cexp", bufs=2, space="PSUM")
        )
        ps_u = ctx.enter_context(tc.tile_pool(name="ps_u", bufs=2, space="PSUM"))
        ps_z = ctx.enter_context(tc.tile_pool(name="ps_z", bufs=2, space="PSUM"))
        pools = (p_pref32, p_pref, p_y, p_w, p_cexp, p_sm, p_u, ps_cexp, ps_u, ps_z)

        LOOKAHEAD = 2
        tiles = {}
        for k in range(min(LOOKAHEAD + 1, NCHUNK)):
            _stage_load(nc, pools, aps, tiles, k)
        for k in range(NCHUNK):
            _build_chunk(nc, pools, consts, aps, tiles, k)
            nxt = k + LOOKAHEAD + 1
            if nxt < NCHUNK:
                _stage_load(nc, pools, aps, tiles, nxt)

    return nc


def _host_consts():
    # SPAIR[p, q] = 1 if q == p//2 else 0   (pair-compress selector)
    sp = np.zeros((128, CHUNK), dtype=np.float16)
    sp[np.arange(128), np.arange(128) // 2] = 1.0
    spT = np.ascontiguousarray(sp.T)
    # fp32 copy is negated: the t_c pair-expansion matmul then yields -t_c,
    # which feeds Abs(t_pref + bias) directly as the fused subtract
    return sp, spT, -spT.astype(np.float32)


_NC_CACHE = None
LAST_RESULT = None


def kernel(pref, c, t_pref, t_c):
    global _NC_CACHE, LAST_RESULT
    if _NC_CACHE is None:
        _NC_CACHE = _build_nc()
    nc = _NC_CACHE

    pref = np.ascontiguousarray(pref, dtype=np.float32)
    c = np.ascontiguousarray(c, dtype=np.float32)
    t_pref = np.ascontiguousarray(t_pref, dtype=np.float32)
    t_c = np.ascontiguousarray(t_c, dtype=np.float32)
    sp, spT, spT32 = _host_consts()

    in_maps = []
    for i in range(NCORES):
        s = slice(i * BPC, (i + 1) * BPC)
        in_maps.append(
            {
                "pref": pref[s],
                "c": c[s],
                "t_pref": t_pref[s],
                "t_c": t_c[s],
                "spair": sp,
                "spairT": spT,
                "spairT32": spT32,
            }
        )

    res = run_bass_kernel_spmd(nc, in_maps, list(range(NCORES)))
    LAST_RESULT = res
    return np.concatenate([r["u"] for r in res.results], axis=0)


# revision 35
# speedup vs baseline: 1.0564x; 1.0564x over previous
"""Trainium2 Bass kernel for the AggregateLayer pooling problem.

reference semantics (per batch b):
    dot_w[j] = <pref[b,j,:], c[b,0,:]>                      (j = 0..63)
    t_w[j]   = 1 / |t_pref[b,0,j] - t_c[b,0]|
    w        = softmax(dot_w + t_w)                          (over j)
    u[b,0,:] = sum_j w[j] * pref[b,j,:]

Strategy: pure data parallel over 8 NeuronCores (1024 batches each).

Per core, batches stream in chunks of 64 (2 MB of fp32 pref). The chunk is
DMA'd CONTIGUOUSLY (16 KB per partition, cast fp32->fp16 in flight by
SWDGE), which puts partition p = 32 consecutive flat rows; with N=64 this
means partition p = (batch pair q=p//2, j-half h=p%2) and free t = j%32.

Engine plan per chunk:
  - GpSimd: pref cast-DMA issue; Y = P16 * c_pair (elementwise); tw chain bits
  - Vector: dots = reduce_sum_d(Y); softmax small ops (pair-merge via
    stream_shuffle); fused weight build W = (e * 1/Z) * SPAIR
  - Scalar: exp (+ per-partition sum accumulator); abs; PSUM->SBUF copies
  - Tensor: c pair-broadcast matmul; 32 accumulating weighted-sum matmuls
    contracting the partition dim with the block-pair selector inside W
No PE transposes and no full-size PSUM->SBUF copies are needed.
"""

import numpy as np
from contextlib import ExitStack

import concourse.bass as bass
import concourse.tile as tile
from concourse import mybir
from concourse.bass_utils import run_bass_kernel_spmd
import concourse.bass2jax as _b2j


def _split_multiwait(bir: dict) -> int:
    """Walrus in this container rejects >1 sync-wait per instruction.

    Hoist excess waits onto NoOps inserted just before the instruction on
    the same engine (program order within the engine stream preserves the
    wait semantics exactly).
    """
    n = 0
    for fn in bir["functions"]:
        for blk in fn["blocks"]:
            out = []
            for inst in blk["instructions"]:
                si = inst.get("sync_info")
                waits = si.get("on_wait") if si else None
                if waits and len(waits) > 1:
                    for w in waits[:-1]:
                        out.append(
                            {
                                "opcode": "NoOp",
                                "engine": inst["engine"],
                                "name": f"{inst['name']}-xw{n}",
                                "ins": [],
                                "outs": [],
                                "sync_info": {"on_update": [], "on_wait": [w]},
                            }
                        )
                        n += 1
                    si["on_wait"] = [waits[-1]]
                out.append(inst)
            blk["instructions"] = out
    return n


_orig_compile_bir_kernel = _b2j.compile_bir_kernel


def _legalizing_compile_bir_kernel(ant_bir_str, *args, **kwargs):
    import orjson

    bir = orjson.loads(ant_bir_str)
    _split_multiwait(bir)
    return _orig_compile_bir_kernel(orjson.dumps(bir), *args, **kwargs)


_b2j.compile_bir_kernel = _legalizing_compile_bir_kernel

F32 = mybir.dt.float32
F16 = mybir.dt.float16
Alu = mybir.AluOpType
Act = mybir.ActivationFunctionType
Axis = mybir.AxisListType

B, N, D = 8192, 64, 128
NCORES = 8
BPC = B // NCORES          # 1024 batches per core
CHUNK = 64                 # batches per chunk
NCHUNK = BPC // CHUNK      # 16
NT = 32                    # free positions per partition row-block (j % 32)
ROWS = CHUNK * N           # 4096 flat rows per chunk

# stream_shuffle mask: swap adjacent partitions within each 32-block
SWAPMASK = [i ^ 1 for i in range(32)]


def _stage_load(nc, pools, aps, tiles, k):
    """Issue chunk k's DMAs + fp32->fp16 cast (runs ahead of compute)."""
    (p_pref32, p_pref, p_y, p_w, p_cexp, p_sm, p_u, ps_cexp, ps_u, ps_z) = pools
    (pref_rows, tp_rows, u_rows) = aps
    r0 = k * ROWS

    p32 = p_pref32.tile([128, NT, D], F32, tag="p32")
    nc.sync.dma_start(
        out=p32[:],
        in_=pref_rows[r0 : r0 + ROWS, :].rearrange("(p t) d -> p t d", p=128),
    )
    # cast writes the (d-half, t, d-low) permuted layout so the dots
    # d-halves fold is a big-descriptor SBUF->SBUF accumulate-DMA
    p16 = p_pref.tile([128, 2, NT, D // 2], F16, tag="p16")
    nc.scalar.copy(
        out=p16[:].transpose([0, 2, 1, 3]),
        in_=p32[:].rearrange("p t (d2 d1) -> p t d2 d1", d2=2),
    )

    tp_k = p_sm.tile([128, NT], F32, tag="tpk")
    nc.sync.dma_start(
        out=tp_k[:],
        in_=tp_rows[k * CHUNK : (k + 1) * CHUNK, :].rearrange(
            "q (h t) -> (q h) t", h=2
        ),
    )
    tiles[k] = (p16, tp_k)


def _phase_a(nc, pools, consts, tiles, k):
    """cexp matmul + Y-mult + fold-DMA issue."""
    (p_pref32, p_pref, p_y, p_w, p_cexp, p_sm, p_u, ps_cexp, ps_u, ps_z) = pools
    (sp16, spT16, cg16, tca) = consts
    (p16, tp_k) = tiles[k]

    cexp_ps = ps_cexp.tile([128, D], F32, tag="cexp_ps")
    nc.tensor.matmul(
        out=cexp_ps[:], lhsT=spT16[:], rhs=cg16[:, k, :], start=True, stop=True
    )
    cexp16 = p_cexp.tile([128, D], F16, tag="cexp16")
    nc.scalar.copy(out=cexp16[:], in_=cexp_ps[:])

    y16 = p_y.tile([128, 2, NT, D // 2], F16, tag="y16")
    nc.vector.tensor_tensor(
        out=y16[:],
        in0=p16[:],
        in1=cexp16[:]
        .rearrange("p (d2 d1) -> p d2 d1", d2=2)
        .unsqueeze(2)
        .broadcast_to((128, 2, NT, D // 2)),
        op=Alu.mult,
    )
    nc.gpsimd.dma_start(
        out=y16[:, 0, :, :], in_=y16[:, 1, :, :], accum_op=Alu.add
    )
    tiles[k] = (p16, tp_k, y16)


def _phase_b(nc, pools, consts, tiles, k):
    """reduce + t_w + -max + pair-merge DMA issue."""
    (p_pref32, p_pref, p_y, p_w, p_cexp, p_sm, p_u, ps_cexp, ps_u, ps_z) = pools
    (sp16, spT16, cg16, tca) = consts
    (p16, tp_k, y16) = tiles[k]

    dots = p_sm.tile([128, NT], F32, tag="dots")
    nc.vector.reduce_sum(out=dots[:], in_=y16[:, 0, :, :], axis=Axis.X)

    adtw = p_sm.tile([128, NT], F32, tag="adtw")
    nc.scalar.activation(
        out=adtw[:], in_=tp_k[:], func=Act.Abs, bias=tca[:, k : k + 1]
    )
    tw = p_sm.tile([128, NT], F32, tag="tw")
    nc.vector.reciprocal(out=tw[:], in_=adtw[:])
    wpre = p_sm.tile([128, NT], F32, tag="wpre")
    nc.vector.tensor_add(out=wpre[:], in0=dots[:], in1=tw[:])
    nmh = p_sm.tile([128, 1], F32, tag="nmh")
    nc.vector.tensor_reduce(
        out=nmh[:], in_=wpre[:], axis=Axis.X, op=Alu.max, negate=True
    )
    nms = p_sm.tile([128, 1], F32, tag="nms")
    nmh_v = nmh[:].rearrange("(q h) one -> q h one", h=2)
    nms_v = nms[:].rearrange("(q h) one -> q h one", h=2)
    nc.sync.dma_start(out=nms_v[:, 0, :], in_=nmh_v[:, 1, :])
    nc.sync.dma_start(out=nms_v[:, 1, :], in_=nmh_v[:, 0, :])
    tiles[k] = (p16, wpre, nmh, nms)


def _phase_c(nc, pools, consts, aps, tiles, k):
    """exp + Z + weight build + weighted sum + store."""
    (p_pref32, p_pref, p_y, p_w, p_cexp, p_sm, p_u, ps_cexp, ps_u, ps_z) = pools
    (sp16, spT16, cg16, tca) = consts
    (pref_rows, tp_rows, u_rows) = aps
    (p16, wpre, nmh, nms) = tiles.pop(k)

    nm = p_sm.tile([128, 1], F32, tag="nm")
    nc.vector.tensor_tensor(out=nm[:], in0=nmh[:], in1=nms[:], op=Alu.min)

    e16 = p_sm.tile([128, NT], F16, tag="e16")
    nc.scalar.activation(
        out=e16[:], in_=wpre[:], func=Act.Exp, bias=nm[:], scale=1.0
    )

    zps = ps_z.tile([CHUNK, NT], F32, tag="zps")
    nc.tensor.matmul(
        out=zps[:], lhsT=sp16[:], rhs=e16[:], start=True, stop=True
    )
    zq = p_sm.tile([CHUNK, 1], F32, tag="zq")
    nc.vector.reduce_sum(out=zq[:], in_=zps[:], axis=Axis.X)
    rzq = p_sm.tile([CHUNK, 1], F32, tag="rzq")
    nc.vector.reciprocal(out=rzq[:], in_=zq[:])

    w16 = p_w.tile([128, NT, CHUNK], F16, tag="w16")
    nc.gpsimd.tensor_tensor(
        out=w16[:],
        in0=e16[:].unsqueeze(2).broadcast_to((128, NT, CHUNK)),
        in1=sp16[:].unsqueeze(1).broadcast_to((128, NT, CHUNK)),
        op=Alu.mult,
    )

    ups = ps_u.tile([CHUNK, D], F32, tag="ups")
    for t in range(NT):
        nc.tensor.matmul(
            out=ups[:],
            lhsT=w16[:, t, :],
            rhs=p16[:, :, t, :],
            start=(t == 0),
            stop=(t == NT - 1),
        )

    usb = p_u.tile([CHUNK, D], F32, tag="usb")
    nc.scalar.mul(out=usb[:], in_=ups[:], mul=rzq[:])
    nc.sync.dma_start(
        out=u_rows[k * CHUNK : (k + 1) * CHUNK, :], in_=usb[:]
    )


def _build_nc():
    nc = bass.Bass()
    pref = nc.declare_dram_parameter("pref", [BPC, N, D], F32, isOutput=False)
    c = nc.declare_dram_parameter("c", [BPC, 1, D], F32, isOutput=False)
    t_pref = nc.declare_dram_parameter("t_pref", [BPC, 1, N], F32, isOutput=False)
    t_c = nc.declare_dram_parameter("t_c", [BPC, 1], F32, isOutput=False)
    spair = nc.declare_dram_parameter("spair", [128, CHUNK], F16, isOutput=False)
    spairT = nc.declare_dram_parameter("spairT", [CHUNK, 128], F16, isOutput=False)
    spairT32 = nc.declare_dram_parameter(
        "spairT32", [CHUNK, 128], F32, isOutput=False
    )
    u = nc.declare_dram_parameter("u", [BPC, 1, D], F32, isOutput=True)

    pref_rows = pref[:].rearrange("b n d -> (b n) d")
    c_rows = c[:].rearrange("b one d -> (b one) d")
    tp_rows = t_pref[:].rearrange("b one n -> (b one) n")
    tc_rows = t_c[:]
    u_rows = u[:].rearrange("b one d -> (b one) d")

    with ExitStack() as ctx:
        tc = ctx.enter_context(tile.TileContext(nc))
        p_const = ctx.enter_context(tc.tile_pool(name="const", bufs=1))

        # constants / preloads (issued up front; small side-queue traffic)
        sp16 = p_const.tile([128, CHUNK], F16)
        nc.sync.dma_start(out=sp16[:], in_=spair[:])
        spT16 = p_const.tile([CHUNK, 128], F16)
        nc.sync.dma_start(out=spT16[:], in_=spairT[:])
        spT32 = p_const.tile([CHUNK, 128], F32)
        nc.sync.dma_start(out=spT32[:], in_=spairT32[:])

        cg16 = p_const.tile([CHUNK, NCHUNK, D], F16)
        nc.gpsimd.dma_start(
            out=cg16[:],
            in_=c_rows[:].rearrange("(k q) d -> q k d", q=CHUNK),
        )
        # t_c on 64 partitions, then expand to partition pairs via PE
        tc64 = p_const.tile([CHUNK, NCHUNK], F32)
        nc.sync.dma_start(
            out=tc64[:],
            in_=tc_rows[:].rearrange("(k q) one -> q (k one)", q=CHUNK),
        )
        ps_t = ctx.enter_context(tc.tile_pool(name="ps_t", bufs=1, space="PSUM"))
        tca_ps = ps_t.tile([128, NCHUNK], F32)
        nc.tensor.matmul(
            out=tca_ps[:], lhsT=spT32[:], rhs=tc64[:], start=True, stop=True
        )
        tca = p_const.tile([128, NCHUNK], F32)
        nc.scalar.copy(out=tca[:], in_=tca_ps[:])

        consts = (sp16, spT16, cg16, tca)
        aps = (pref_rows, tp_rows, u_rows)

        p_pref32 = ctx.enter_context(tc.tile_pool(name="pref32", bufs=3))
        p_pref = ctx.enter_context(tc.tile_pool(name="pref", bufs=4))
        p_y = ctx.enter_context(tc.tile_pool(name="y", bufs=3))
        p_w = ctx.enter_context(tc.tile_pool(name="w", bufs=3))
        p_cexp = ctx.enter_context(tc.tile_pool(name="cexp", bufs=3))
        p_sm = ctx.enter_context(tc.tile_pool(name="sm", bufs=6))
        p_u = ctx.enter_context(tc.tile_pool(name="u", bufs=3))
        ps_cexp = ctx.enter_context(
            tc.tile_pool(name="ps_cexp", bufs=2, space="PSUM")
        )
        ps_u = ctx.enter_context(tc.tile_pool(name="ps_u", bufs=2, space="PSUM"))
        ps_z = ctx.enter_context(tc.tile_pool(name="ps_z", bufs=2, space="PSUM"))
        pools = (p_pref32, p_pref, p_y, p_w, p_cexp, p_sm, p_u, ps_cexp, ps_u, ps_z)

        LOOKAHEAD = 3
        tiles = {}
        for k in range(min(LOOKAHEAD, NCHUNK)):
            _stage_load(nc, pools, aps, tiles, k)
        for i in range(NCHUNK + 2):
            nxt = i + LOOKAHEAD
            if nxt < NCHUNK:
                _stage_load(nc, pools, aps, tiles, nxt)
            if i < NCHUNK:
                _phase_a(nc, pools, consts, tiles, i)
            if 1 <= i + 0 and i - 1 >= 0 and i - 1 < NCHUNK:
                _phase_b(nc, pools, consts, tiles, i - 1)
            if i - 2 >= 0:
                _phase_c(nc, pools, consts, aps, tiles, i - 2)

    return nc


def _host_consts():
    # SPAIR[p, q] = 1 if q == p//2 else 0   (pair-compress selector)
    sp = np.zeros((128, CHUNK), dtype=np.float16)
    sp[np.arange(128), np.arange(128) // 2] = 1.0
    spT = np.ascontiguousarray(sp.T)
    # fp32 copy is negated: the t_c pair-expansion matmul then yields -t_c,
    # which feeds Abs(t_pref + bias) directly as the fused subtract
    return sp, spT, -spT.astype(np.float32)


_NC_CACHE = None
LAST_RESULT = None


def kernel(pref, c, t_pref, t_c):
    global _NC_CACHE, LAST_RESULT
    if _NC_CACHE is None:
        _NC_CACHE = _build_nc()
    nc = _NC_CACHE

    pref = np.ascontiguousarray(pref, dtype=np.float32)
    c = np.ascontiguousarray(c, dtype=np.float32)
    t_pref = np.ascontiguousarray(t_pref, dtype=np.float32)
    t_c = np.ascontiguousarray(t_c, dtype=np.float32)
    sp, spT, spT32 = _host_consts()

    in_maps = []
    for i in range(NCORES):
        s = slice(i * BPC, (i + 1) * BPC)
        in_maps.append(
            {
                "pref": pref[s],
                "c": c[s],
                "t_pref": t_pref[s],
                "t_c": t_c[s],
                "spair": sp,
                "spairT": spT,
                "spairT32": spT32,
            }
        )

    res = run_bass_kernel_spmd(nc, in_maps, list(range(NCORES)))
    LAST_RESULT = res
    return np.concatenate([r["u"] for r in res.results], axis=0)


# revision 36
# speedup vs baseline: 1.1004x; 1.0416x over previous
"""Trainium2 Bass kernel for the AggregateLayer pooling problem.

reference semantics (per batch b):
    dot_w[j] = <pref[b,j,:], c[b,0,:]>                      (j = 0..63)
    t_w[j]   = 1 / |t_pref[b,0,j] - t_c[b,0]|
    w        = softmax(dot_w + t_w)                          (over j)
    u[b,0,:] = sum_j w[j] * pref[b,j,:]

Strategy: pure data parallel over 8 NeuronCores (1024 batches each).

Per core, batches stream in chunks of 64 (2 MB of fp32 pref). The chunk is
DMA'd CONTIGUOUSLY (16 KB per partition, cast fp32->fp16 in flight by
SWDGE), which puts partition p = 32 consecutive flat rows; with N=64 this
means partition p = (batch pair q=p//2, j-half h=p%2) and free t = j%32.

Engine plan per chunk:
  - GpSimd: pref cast-DMA issue; Y = P16 * c_pair (elementwise); tw chain bits
  - Vector: dots = reduce_sum_d(Y); softmax small ops (pair-merge via
    stream_shuffle); fused weight build W = (e * 1/Z) * SPAIR
  - Scalar: exp (+ per-partition sum accumulator); abs; PSUM->SBUF copies
  - Tensor: c pair-broadcast matmul; 32 accumulating weighted-sum matmuls
    contracting the partition dim with the block-pair selector inside W
No PE transposes and no full-size PSUM->SBUF copies are needed.
"""

import numpy as np
from contextlib import ExitStack

import concourse.bass as bass
import concourse.tile as tile
from concourse import mybir
from concourse.bass_utils import run_bass_kernel_spmd
import concourse.bass2jax as _b2j


def _split_multiwait(bir: dict) -> int:
    """Walrus in this container rejects >1 sync-wait per instruction.

    Hoist excess waits onto NoOps inserted just before the instruction on
    the same engine (program order within the engine stream preserves the
    wait semantics exactly).
    """
    n = 0
    for fn in bir["functions"]:
        for blk in fn["blocks"]:
            out = []
            for inst in blk["instructions"]:
                si = inst.get("sync_info")
                waits = si.get("on_wait") if si else None
                if waits and len(waits) > 1:
                    for w in waits[:-1]:
                        out.append(
                            {
                                "opcode": "NoOp",
                                "engine": inst["engine"],
                                "name": f"{inst['name']}-xw{n}",
                                "ins": [],
                                "outs": [],
                                "sync_info": {"on_update": [], "on_wait": [w]},
                            }
                        )
                        n += 1
                    si["on_wait"] = [waits[-1]]
                out.append(inst)
            blk["instructions"] = out
    return n


_orig_compile_bir_kernel = _b2j.compile_bir_kernel


def _legalizing_compile_bir_kernel(ant_bir_str, *args, **kwargs):
    import orjson

    bir = orjson.loads(ant_bir_str)
    _split_multiwait(bir)
    return _orig_compile_bir_kernel(orjson.dumps(bir), *args, **kwargs)


_b2j.compile_bir_kernel = _legalizing_compile_bir_kernel

F32 = mybir.dt.float32
F16 = mybir.dt.float16
Alu = mybir.AluOpType
Act = mybir.ActivationFunctionType
Axis = mybir.AxisListType

B, N, D = 8192, 64, 128
NCORES = 8
BPC = B // NCORES          # 1024 batches per core
CHUNK = 64                 # batches per chunk
NCHUNK = BPC // CHUNK      # 16
NT = 32                    # free positions per partition row-block (j % 32)
ROWS = CHUNK * N           # 4096 flat rows per chunk

# stream_shuffle mask: swap adjacent partitions within each 32-block
SWAPMASK = [i ^ 1 for i in range(32)]


def _stage_load(nc, pools, aps, tiles, k):
    """Issue chunk k's DMAs + fp32->fp16 cast (runs ahead of compute)."""
    (p_pref32, p_pref, p_y, p_w, p_cexp, p_sm, p_u, ps_cexp, ps_u, ps_z) = pools
    (pref_rows, tp_rows, u_rows) = aps
    r0 = k * ROWS

    p32 = p_pref32.tile([128, NT, D], F32, tag="p32")
    nc.sync.dma_start(
        out=p32[:],
        in_=pref_rows[r0 : r0 + ROWS, :].rearrange("(p t) d -> p t d", p=128),
    )
    # cast writes the (d-half, t, d-low) permuted layout so the dots
    # d-halves fold is a big-descriptor SBUF->SBUF accumulate-DMA
    p16 = p_pref.tile([128, 2, NT, D // 2], F16, tag="p16")
    nc.scalar.copy(
        out=p16[:].transpose([0, 2, 1, 3]),
        in_=p32[:].rearrange("p t (d2 d1) -> p t d2 d1", d2=2),
    )

    tp_k = p_sm.tile([128, NT], F32, tag="tpk")
    nc.sync.dma_start(
        out=tp_k[:],
        in_=tp_rows[k * CHUNK : (k + 1) * CHUNK, :].rearrange(
            "q (h t) -> (q h) t", h=2
        ),
    )
    tiles[k] = (p16, tp_k)


def _phase_a(nc, pools, consts, tiles, k):
    """cexp matmul + Y-mult + fold-DMA issue."""
    (p_pref32, p_pref, p_y, p_w, p_cexp, p_sm, p_u, ps_cexp, ps_u, ps_z) = pools
    (sp16, spT16, cg16, tca) = consts
    (p16, tp_k) = tiles[k]

    cexp_ps = ps_cexp.tile([128, D], F32, tag="cexp_ps")
    nc.tensor.matmul(
        out=cexp_ps[:], lhsT=spT16[:], rhs=cg16[:, k, :], start=True, stop=True
    )
    cexp16 = p_cexp.tile([128, D], F16, tag="cexp16")
    nc.scalar.copy(out=cexp16[:], in_=cexp_ps[:])

    y16 = p_y.tile([128, 2, NT, D // 2], F16, tag="y16")
    nc.vector.tensor_tensor(
        out=y16[:],
        in0=p16[:],
        in1=cexp16[:]
        .rearrange("p (d2 d1) -> p d2 d1", d2=2)
        .unsqueeze(2)
        .broadcast_to((128, 2, NT, D // 2)),
        op=Alu.mult,
    )
    nc.gpsimd.dma_start(
        out=y16[:, 0, :, :], in_=y16[:, 1, :, :], accum_op=Alu.add
    )
    tiles[k] = (p16, tp_k, y16)


def _phase_b(nc, pools, consts, tiles, k):
    """reduce + t_w + -max + pair-merge DMA issue."""
    (p_pref32, p_pref, p_y, p_w, p_cexp, p_sm, p_u, ps_cexp, ps_u, ps_z) = pools
    (sp16, spT16, cg16, tca) = consts
    (p16, tp_k, y16) = tiles[k]

    dots = p_sm.tile([128, NT], F32, tag="dots")
    nc.vector.reduce_sum(out=dots[:], in_=y16[:, 0, :, :], axis=Axis.X)

    adtw = p_sm.tile([128, NT], F32, tag="adtw")
    nc.scalar.activation(
        out=adtw[:], in_=tp_k[:], func=Act.Abs, bias=tca[:, k : k + 1]
    )
    tw = p_sm.tile([128, NT], F32, tag="tw")
    nc.vector.reciprocal(out=tw[:], in_=adtw[:])
    wpre = p_sm.tile([128, NT], F32, tag="wpre")
    nc.vector.tensor_add(out=wpre[:], in0=dots[:], in1=tw[:])
    nmh = p_sm.tile([128, 1], F32, tag="nmh")
    nc.vector.tensor_reduce(
        out=nmh[:], in_=wpre[:], axis=Axis.X, op=Alu.max, negate=True
    )
    nms = p_sm.tile([128, 1], F32, tag="nms")
    nmh_v = nmh[:].rearrange("(q h) one -> q h one", h=2)
    nms_v = nms[:].rearrange("(q h) one -> q h one", h=2)
    nc.sync.dma_start(out=nms_v[:, 0, :], in_=nmh_v[:, 1, :])
    nc.sync.dma_start(out=nms_v[:, 1, :], in_=nmh_v[:, 0, :])
    tiles[k] = (p16, wpre, nmh, nms)


def _phase_c(nc, pools, consts, aps, tiles, k):
    """exp + Z + weight build + weighted sum + store."""
    (p_pref32, p_pref, p_y, p_w, p_cexp, p_sm, p_u, ps_cexp, ps_u, ps_z) = pools
    (sp16, spT16, cg16, tca) = consts
    (pref_rows, tp_rows, u_rows) = aps
    (p16, wpre, nmh, nms) = tiles.pop(k)

    nm = p_sm.tile([128, 1], F32, tag="nm")
    nc.vector.tensor_tensor(out=nm[:], in0=nmh[:], in1=nms[:], op=Alu.min)

    e16 = p_sm.tile([128, NT], F16, tag="e16")
    nc.scalar.activation(
        out=e16[:], in_=wpre[:], func=Act.Exp, bias=nm[:], scale=1.0
    )

    zps = ps_z.tile([CHUNK, NT], F32, tag="zps")
    nc.tensor.matmul(
        out=zps[:], lhsT=sp16[:], rhs=e16[:], start=True, stop=True
    )
    zq = p_sm.tile([CHUNK, 1], F32, tag="zq")
    nc.vector.reduce_sum(out=zq[:], in_=zps[:], axis=Axis.X)
    rzq = p_sm.tile([CHUNK, 1], F32, tag="rzq")
    nc.vector.reciprocal(out=rzq[:], in_=zq[:])

    w16 = p_w.tile([128, NT, CHUNK], F16, tag="w16")
    nc.gpsimd.tensor_tensor(
        out=w16[:],
        in0=e16[:].unsqueeze(2).broadcast_to((128, NT, CHUNK)),
        in1=sp16[:].unsqueeze(1).broadcast_to((128, NT, CHUNK)),
        op=Alu.mult,
    )

    ups = ps_u.tile([CHUNK, D], F32, tag="ups")
    for t in range(NT):
        nc.tensor.matmul(
            out=ups[:],
            lhsT=w16[:, t, :],
            rhs=p16[:, :, t, :],
            start=(t == 0),
            stop=(t == NT - 1),
        )

    usb = p_u.tile([CHUNK, D], F32, tag="usb")
    nc.scalar.mul(out=usb[:], in_=ups[:], mul=rzq[:])
    nc.sync.dma_start(
        out=u_rows[k * CHUNK : (k + 1) * CHUNK, :], in_=usb[:]
    )


def _build_nc():
    nc = bass.Bass()
    pref = nc.declare_dram_parameter("pref", [BPC, N, D], F32, isOutput=False)
    c = nc.declare_dram_parameter("c", [BPC, 1, D], F32, isOutput=False)
    t_pref = nc.declare_dram_parameter("t_pref", [BPC, 1, N], F32, isOutput=False)
    t_c = nc.declare_dram_parameter("t_c", [BPC, 1], F32, isOutput=False)
    spair = nc.declare_dram_parameter("spair", [128, CHUNK], F16, isOutput=False)
    spairT = nc.declare_dram_parameter("spairT", [CHUNK, 128], F16, isOutput=False)
    spairT32 = nc.declare_dram_parameter(
        "spairT32", [CHUNK, 128], F32, isOutput=False
    )
    u = nc.declare_dram_parameter("u", [BPC, 1, D], F32, isOutput=True)

    pref_rows = pref[:].rearrange("b n d -> (b n) d")
    c_rows = c[:].rearrange("b one d -> (b one) d")
    tp_rows = t_pref[:].rearrange("b one n -> (b one) n")
    tc_rows = t_c[:]
    u_rows = u[:].rearrange("b one d -> (b one) d")

    with ExitStack() as ctx:
        tc = ctx.enter_context(tile.TileContext(nc))
        p_const = ctx.enter_context(tc.tile_pool(name="const", bufs=1))

        # constants / preloads (issued up front; small side-queue traffic)
        sp16 = p_const.tile([128, CHUNK], F16)
        nc.sync.dma_start(out=sp16[:], in_=spair[:])
        spT16 = p_const.tile([CHUNK, 128], F16)
        nc.sync.dma_start(out=spT16[:], in_=spairT[:])
        spT32 = p_const.tile([CHUNK, 128], F32)
        nc.sync.dma_start(out=spT32[:], in_=spairT32[:])

        cg16 = p_const.tile([CHUNK, NCHUNK, D], F16)
        nc.gpsimd.dma_start(
            out=cg16[:],
            in_=c_rows[:].rearrange("(k q) d -> q k d", q=CHUNK),
        )
        # t_c on 64 partitions, then expand to partition pairs via PE
        tc64 = p_const.tile([CHUNK, NCHUNK], F32)
        nc.sync.dma_start(
            out=tc64[:],
            in_=tc_rows[:].rearrange("(k q) one -> q (k one)", q=CHUNK),
        )
        ps_t = ctx.enter_context(tc.tile_pool(name="ps_t", bufs=1, space="PSUM"))
        tca_ps = ps_t.tile([128, NCHUNK], F32)
        nc.tensor.matmul(
            out=tca_ps[:], lhsT=spT32[:], rhs=tc64[:], start=True, stop=True
        )
        tca = p_const.tile([128, NCHUNK], F32)
        nc.scalar.copy(out=tca[:], in_=tca_ps[:])

        consts = (sp16, spT16, cg16, tca)
        aps = (pref_rows, tp_rows, u_rows)

        p_pref32 = ctx.enter_context(tc.tile_pool(name="pref32", bufs=3))
        p_pref = ctx.enter_context(tc.tile_pool(name="pref", bufs=7))
        p_y = ctx.enter_context(tc.tile_pool(name="y", bufs=4))
        p_w = ctx.enter_context(tc.tile_pool(name="w", bufs=3))
        p_cexp = ctx.enter_context(tc.tile_pool(name="cexp", bufs=3))
        p_sm = ctx.enter_context(tc.tile_pool(name="sm", bufs=8))
        p_u = ctx.enter_context(tc.tile_pool(name="u", bufs=3))
        ps_cexp = ctx.enter_context(
            tc.tile_pool(name="ps_cexp", bufs=2, space="PSUM")
        )
        ps_u = ctx.enter_context(tc.tile_pool(name="ps_u", bufs=2, space="PSUM"))
        ps_z = ctx.enter_context(tc.tile_pool(name="ps_z", bufs=2, space="PSUM"))
        pools = (p_pref32, p_pref, p_y, p_w, p_cexp, p_sm, p_u, ps_cexp, ps_u, ps_z)

        LOOKAHEAD = 3
        tiles = {}
        for k in range(min(LOOKAHEAD, NCHUNK)):
            _stage_load(nc, pools, aps, tiles, k)
        for i in range(NCHUNK + 2):
            nxt = i + LOOKAHEAD
            if nxt < NCHUNK:
                _stage_load(nc, pools, aps, tiles, nxt)
            if i < NCHUNK:
                _phase_a(nc, pools, consts, tiles, i)
            if 1 <= i + 0 and i - 1 >= 0 and i - 1 < NCHUNK:
                _phase_b(nc, pools, consts, tiles, i - 1)
            if i - 2 >= 0:
                _phase_c(nc, pools, consts, aps, tiles, i - 2)

    return nc


def _host_consts():
    # SPAIR[p, q] = 1 if q == p//2 else 0   (pair-compress selector)
    sp = np.zeros((128, CHUNK), dtype=np.float16)
    sp[np.arange(128), np.arange(128) // 2] = 1.0
    spT = np.ascontiguousarray(sp.T)
    # fp32 copy is negated: the t_c pair-expansion matmul then yields -t_c,
    # which feeds Abs(t_pref + bias) directly as the fused subtract
    return sp, spT, -spT.astype(np.float32)


_NC_CACHE = None
LAST_RESULT = None


def kernel(pref, c, t_pref, t_c):
    global _NC_CACHE, LAST_RESULT
    if _NC_CACHE is None:
        _NC_CACHE = _build_nc()
    nc = _NC_CACHE

    pref = np.ascontiguousarray(pref, dtype=np.float32)
    c = np.ascontiguousarray(c, dtype=np.float32)
    t_pref = np.ascontiguousarray(t_pref, dtype=np.float32)
    t_c = np.ascontiguousarray(t_c, dtype=np.float32)
    sp, spT, spT32 = _host_consts()

    in_maps = []
    for i in range(NCORES):
        s = slice(i * BPC, (i + 1) * BPC)
        in_maps.append(
            {
                "pref": pref[s],
                "c": c[s],
                "t_pref": t_pref[s],
                "t_c": t_c[s],
                "spair": sp,
                "spairT": spT,
                "spairT32": spT32,
            }
        )

    res = run_bass_kernel_spmd(nc, in_maps, list(range(NCORES)))
    LAST_RESULT = res
    return np.concatenate([r["u"] for r in res.results], axis=0)
